# revision 67
# baseline (speedup 1.0000x reference)
"""LCSA (local convolutional sparse attention) Trainium2 Bass kernel.

Problem: B=2, S=2048, D=1024, H=8 heads, E=128 head width, KW=16 kernel width,
per-head dilations [1,1,2,2,4,4,8,8].

Sharding: data-parallel over (batch, sequence): core c handles batch c//4,
sequence chunk (c%4)*512..+512, with a 64-token zero-padded halo per side.

Device algorithm per core (fp16 q/k path, bf16 value path, fp32 accum):
  - qT[h] = Wq[h].T @ xT [E,512]; kT[h] [E, per-head trimmed span] (fp16 on
    PE, fp32 PSUM).  k-bias dropped (uniform per-query logit shift -> softmax
    invariant); q-bias via ACT copy; kT edges outside the reachable span are
    zeroed once so masked logits stay finite.
  - v = xTb.T @ Wv_allheads [640, H*E] in bf16 (xt cast to bf16 on Pool).
    v-bias and out-bias folded into a host-side constant (sum of scores = 1).
  - Per (query tile i, head h): PSUM logits = mask (PE identity-matmul
    preload, in-window value -40 to bound exp) + qT_i.T @ kT window [128,256];
    exp+rowsum on ACT (no max subtraction; |logit|<=81 so exp(l-40) is safe);
    reciprocal on DVE; normalize on Pool (bf16); transpose via PE (bf16);
    attnT = v.T @ scoreT (bf16); out_i = sum_h attnT.T @ Wo[h] (bf16 moving).
  - Software-pipelined emission (5-deep slots) keeps PE ~99% busy through the
    attention phase; warm-up matmuls ramp the PE p-state before data lands.
"""

import numpy as np

B, S, D, H, E, KW = 2, 2048, 1024, 8, 128, 16
HALO = 64          # covers max offset d*(KW-1)//2 = 60 for d=8
CHUNK = 512        # query tokens per core
SPAN = CHUNK + 2 * HALO   # 640 kv tokens per core
NST = SPAN // 128  # 5 sequence tiles
NQT = CHUNK // 128 # 4 query tiles
NC_ = 8            # cores
DC = D // 128      # 8 contraction chunks
NT = NQT * H       # 32 attention tiles per core
MASKVAL = -30000.0
SHIFT = -40.0      # in-window logit shift; bounds exp while leaving softmax exact

_CACHE: dict = {}
N_WARM = 8         # PE warm-up matmuls (p-state ramp + DMA-latency cover)
SEAM_FILL = 4      # fillers at phase-1 seams to bridge DMA waits (p-state)
QK0_FILL = 1        # filler matmuls between head-0 q chunks (xt DMA pacing)
DILATIONS = (1, 1, 2, 2, 4, 4, 8, 8)
# per-head kv span (in 640-wide span coords) actually reachable by the windows
K_SPANS = tuple((HALO - (15 * d) // 2, HALO + CHUNK + 15 * d - (15 * d) // 2)
                for d in DILATIONS)
# per-head logits window width from 128*i (span coords), multiple of 8, <=256
W_H = tuple(min(256, (HALO + 128 + 15 * d - (15 * d) // 2 + 7) // 8 * 8)
            for d in DILATIONS)


def _build_nc(reps=1, f32r=True):
    from contextlib import ExitStack

    import concourse.bacc as bacc
    import concourse.tile as tile
    from concourse import mybir
    from concourse.masks import make_identity

    F32 = mybir.dt.float32
    BF16 = mybir.dt.bfloat16
    FP16 = mybir.dt.float16
    FR = mybir.dt.float32r if f32r else F32
    AF = mybir.ActivationFunctionType

    nc = bacc.Bacc("TRN2", target_bir_lowering=False, debug=False, num_devices=1)

    # q/k path in fp16 (4x finer mantissa than bf16; halves the weight DMA).
    # wq/wk pre-rearranged on host to [H, 128, DC*E] so DMA rows stay >=512B.
    xt_d = nc.dram_tensor("xt", [D, SPAN], FP16, kind="ExternalInput").ap()
    wq_d = nc.dram_tensor("wq", [H, 128, DC * E], FP16, kind="ExternalInput").ap()
    wk_d = nc.dram_tensor("wk", [H, 128, DC * E], FP16, kind="ExternalInput").ap()
    wvrb_d = nc.dram_tensor("wvrb", [D, H * E], BF16, kind="ExternalInput").ap()
    wosb_d = nc.dram_tensor("wosb", [H, E, D], BF16, kind="ExternalInput").ap()
    mkb_d = nc.dram_tensor("mkb", [H, 128, 256], BF16, kind="ExternalInput").ap()
    bqt_d = nc.dram_tensor("bqt", [E, H], F32, kind="ExternalInput").ap()
    out_d = nc.dram_tensor("out", [CHUNK, D], F32, kind="ExternalOutput").ap()

    with tile.TileContext(nc) as tc, ExitStack() as ctx:
        const_p = ctx.enter_context(tc.tile_pool(name="const", bufs=1))
        big_s = ctx.enter_context(tc.tile_pool(name="bigs", bufs=1))
        wring = ctx.enter_context(tc.tile_pool(name="wring", bufs=4))
        sm_p = ctx.enter_context(tc.tile_pool(name="sm", bufs=4))
        smv_p = ctx.enter_context(tc.tile_pool(name="smv", bufs=5))
        ob_p = ctx.enter_context(tc.tile_pool(name="ob", bufs=2))
        # PSUM: 8 banks exactly: big(3) + lg(3) + st(1) + at(1)
        ps_big = ctx.enter_context(tc.tile_pool(name="ps_big", bufs=4, space="PSUM"))
        ps_lg = ctx.enter_context(tc.tile_pool(name="ps_lg", bufs=2, space="PSUM"))
        ps_st = ctx.enter_context(tc.tile_pool(name="ps_st", bufs=1, space="PSUM"))
        ps_at = ctx.enter_context(tc.tile_pool(name="ps_at", bufs=1, space="PSUM"))

        # ---- constants (Pool-generated; no DMA dependency) ----
        warmb = const_p.tile([128, 256], BF16)
        nc.gpsimd.memset(warmb, 0.0)
        identb = const_p.tile([128, 128], BF16)
        make_identity(nc, identb)

        for _rep in range(reps):
            _emit(nc, tc, mybir, F32, BF16, FP16, FR, AF,
                  xt_d, wq_d, wk_d, wvrb_d, wosb_d, mkb_d, bqt_d, out_d,
                  const_p, big_s, wring, sm_p, smv_p, ob_p,
                  ps_big, ps_lg, ps_st, ps_at, identb, warmb)

    nc.compile()
    return nc


def _emit(nc, tc, mybir, F32, BF16, FP16, FR, AF,
          xt_d, wq_d, wk_d, wvrb_d, wosb_d, mkb_d, bqt_d, out_d,
          const_p, big_s, wring, sm_p, smv_p, ob_p,
          ps_big, ps_lg, ps_st, ps_at, identb, warmb):
    # ---- resident loads, ordered by first PE use; head-0 weights and xt are
    # chunk-interleaved so the first q-projection matmul can start ~2.5us in ----
    wq0 = wring.tile([128, DC, E], FP16, tag="wq", name="wq0")
    wk0 = wring.tile([128, DC, E], FP16, tag="wk", name="wk0")
    xt_sb = big_s.tile([128, DC, SPAN], FP16, tag="xt")
    wvrb_sb = big_s.tile([128, DC, H * E], BF16, tag="wvrb")
    nc.sync.dma_start(xt_sb[:, 0, :], xt_d[0:128, :])
    nc.sync.dma_start(wvrb_sb[:, :, 0:512],
                      wvrb_d[:, 0:512].rearrange("(c p) n -> p c n", p=128))
    for c in range(1, DC):
        nc.sync.dma_start(xt_sb[:, c, :], xt_d[128 * c:128 * (c + 1), :])
    nc.sync.dma_start(wq0, wq_d[0].rearrange("p (c e) -> p c e", c=DC))
    nc.sync.dma_start(wk0, wk_d[0].rearrange("p (c e) -> p c e", c=DC))
    bqt_sb = big_s.tile([128, H], F32, tag="bqt")
    nc.sync.dma_start(bqt_sb, bqt_d)

    w_ring = {0: (wq0, wk0)}
    def _load_head(h):
        wqh = wring.tile([128, DC, E], FP16, tag="wq", name=f"wq{h}")
        nc.sync.dma_start(wqh, wq_d[h].rearrange("p (c e) -> p c e", c=DC))
        wkh = wring.tile([128, DC, E], FP16, tag="wk", name=f"wk{h}")
        nc.sync.dma_start(wkh, wk_d[h].rearrange("p (c e) -> p c e", c=DC))
        w_ring[h] = (wqh, wkh)

    _load_head(1)
    nc.sync.dma_start(wvrb_sb[:, :, 512:1024],
                      wvrb_d[:, 512:1024].rearrange("(c p) n -> p c n", p=128))
    _load_head(2)
    _load_head(3)
    mkb_sb = big_s.tile([128, H, 256], BF16, tag="mkb")
    nc.sync.dma_start(mkb_sb, mkb_d.rearrange("h p t -> p h t"))
    wosb_sb = big_s.tile([128, H, D], BF16, tag="wosb")
    nc.sync.dma_start(wosb_sb, wosb_d.rearrange("h e d -> e h d"))
    for h in range(4, H):
        _load_head(h)

    # ---- persistent projection outputs ----
    qT_sb = big_s.tile([128, H, CHUNK], FP16, tag="qT")  # [e, h, s]
    kT_sb = big_s.tile([128, H, SPAN], FP16, tag="kT")   # [e, h, s]
    xtb_sb = big_s.tile([128, DC, SPAN], BF16, tag="xtb")
    vb_sb = big_s.tile([128, NST, H * E], BF16, tag="vb")  # [s, tile, h*E+e]

    # ---- PE warm-up: ramp p-state while DMAs stream (no data deps) ----
    warm_n = [0]
    def _warm(k):
        for _ in range(k):
            wp = ps_lg.tile([128, 256], F32, tag="lg", name=f"warm{warm_n[0]}")
            warm_n[0] += 1
            nc.tensor.matmul(wp, warmb[:, 0:128], warmb[:, 0:256],
                             start=True, stop=True)

    _warm(N_WARM)

    # ---- Pool setup: zero kT (edges beyond K_SPANS must be finite), cast xt
    # to bf16 for the v projection ----
    nc.gpsimd.memset(kT_sb, 0.0)
    for c in range(DC):
        nc.vector.tensor_copy(xtb_sb[:, c, :], xt_sb[:, c, :])

    # ---- phase 1: projections ----
    def _qk(h, fill=0):
        wqh, wkh = w_ring[h]
        qp = ps_big.tile([128, 512], F32, tag="big", name=f"qp{h}")
        for c in range(DC):
            nc.tensor.matmul(qp, wqh[:, c, :], xt_sb[:, c, HALO:HALO + CHUNK],
                             start=(c == 0), stop=(c == DC - 1))
            if c < DC - 1:
                _warm(fill)  # cover DMA-paced gaps while xt chunks stream in
        nc.scalar.activation(qT_sb[:, h, :], qp, mybir.ActivationFunctionType.Identity,
                             bias=bqt_sb[:, h:h + 1], scale=1.0)
        # k projected only over the span this head's dilated windows can touch;
        # the rest of kT stays at the one-time memset zeros (masked out anyway)
        s0, s1 = K_SPANS[h]
        w1 = (s1 - s0) // 2
        for sl in (slice(s0, s0 + w1), slice(s0 + w1, s1)):
            kp = ps_big.tile([128, 512], F32, tag="big", name=f"kp{h}_{sl.start}")
            w = sl.stop - sl.start
            for c in range(DC):
                nc.tensor.matmul(kp[:, 0:w], wkh[:, c, :], xt_sb[:, c, sl],
                                 start=(c == 0), stop=(c == DC - 1))
            nc.scalar.copy(kT_sb[:, h, sl], kp[:, 0:w])

    def _vhalf0_chunkmajor():
        # consume xt chunks as they arrive: 4 open accumulators (j=0..3)
        vps = []
        for j in range(4):
            vp = ps_big.tile([128, 512], F32, tag="big", name=f"vp0_{j}")
            vps.append(vp)
        for c in range(DC):
            for j in range(4):
                nc.tensor.matmul(vps[j], xtb_sb[:, c, 128 * j:128 * (j + 1)],
                                 wvrb_sb[:, c, 0:512],
                                 start=(c == 0), stop=(c == DC - 1))
        for j in range(4):
            nc.vector.tensor_copy(vb_sb[:, j, 0:512], vps[j])
        vp4 = ps_big.tile([128, 512], F32, tag="big", name="vp0_4")
        for c in range(DC):
            nc.tensor.matmul(vp4, xtb_sb[:, c, 512:640], wvrb_sb[:, c, 0:512],
                             start=(c == 0), stop=(c == DC - 1))
        nc.vector.tensor_copy(vb_sb[:, 4, 0:512], vp4)

    def _vhalf(half):
        nsl = slice(512 * half, 512 * (half + 1))
        for j in range(NST):
            vp = ps_big.tile([128, 512], F32, tag="big", name=f"vp{half}_{j}")
            for c in range(DC):
                nc.tensor.matmul(vp, xtb_sb[:, c, 128 * j:128 * (j + 1)],
                                 wvrb_sb[:, c, nsl], start=(c == 0), stop=(c == DC - 1))
            nc.vector.tensor_copy(vb_sb[:, j, nsl], vp)

    # ---- phase 2 closures: attention, software pipelined ----
    lg_t, ex_t, se_t, rc_t, sc_t, st_t, sct_t, at_t, ats_t = ({} for _ in range(9))
    ou_t = {}

    def e_lg(t):
        i, h = divmod(t, 8)
        w = W_H[h]
        lg = ps_lg.tile([128, 256], F32, tag="lg", name=f"lg{t}")
        lg_t[t] = lg
        nc.tensor.matmul(lg[:, 0:w], identb, mkb_sb[:, h, 0:w],
                         start=True, stop=False)
        nc.tensor.matmul(lg[:, 0:w], qT_sb[:, h, 128 * i:128 * (i + 1)],
                         kT_sb[:, h, 128 * i:128 * i + w],
                         start=False, stop=True)

    def e_exp(t):
        ex = sm_p.tile([128, 256], BF16, tag="ex", name=f"ex{t}")
        se = smv_p.tile([128, 1], F32, tag="se", name=f"se{t}")
        w = W_H[t % 8]
        nc.scalar.activation(ex[:, 0:w], lg_t.pop(t)[:, 0:w], AF.Exp,
                             bias=0.0, scale=1.0, accum_out=se)
        ex_t[t], se_t[t] = ex, se

    def e_recip(t):
        rc = smv_p.tile([128, 1], F32, tag="rc", name=f"rc{t}")
        nc.vector.reciprocal(rc, se_t.pop(t))
        rc_t[t] = rc

    def e_mul(t):
        sc = sm_p.tile([128, 256], BF16, tag="sc", name=f"sc{t}")
        w = W_H[t % 8]
        nc.gpsimd.tensor_scalar_mul(sc[:, 0:w], ex_t.pop(t)[:, 0:w], rc_t.pop(t))
        sc_t[t] = sc

    def e_tr(t):
        w = W_H[t % 8]
        st = ps_st.tile([128, 256], BF16, tag="st", name=f"st{t}")
        if t == 0:
            # one-time init: the full-width sct copy below may read the
            # (never-transposed) corner of this single-buffer ring
            nc.tensor.transpose(st[:, 128:256], warmb[:, 0:128], identb)
        sc = sc_t.pop(t)
        nc.tensor.transpose(st[:, 0:128], sc[:, 0:128], identb)
        nc.tensor.transpose(st[0:w - 128, 128:256], sc[:, 128:w], identb)
        st_t[t] = st

    def e_sct(t):
        sct = sm_p.tile([128, 256], BF16, tag="sct", name=f"sct{t}")
        nc.vector.tensor_copy(sct, st_t.pop(t))
        sct_t[t] = sct

    def e_at(t):
        i, h = divmod(t, 8)
        w = W_H[h]
        at = ps_at.tile([128, 128], F32, tag="at", name=f"at{t}")
        sct = sct_t.pop(t)
        nc.tensor.matmul(at, vb_sb[:, i, E * h:E * (h + 1)], sct[:, 0:128],
                         start=True, stop=False)
        nc.tensor.matmul(at, vb_sb[0:w - 128, i + 1, E * h:E * (h + 1)],
                         sct[0:w - 128, 128:256], start=False, stop=True)
        at_t[t] = at

    def e_ats(t):
        ats = sm_p.tile([128, 128], BF16, tag="ats", name=f"ats{t}")
        nc.vector.tensor_copy(ats, at_t.pop(t))
        ats_t[t] = ats

    def e_op(t):
        i, h = divmod(t, 8)
        if h == 0:
            ou0 = ps_big.tile([128, 512], F32, tag="big", name=f"ou0_{i}")
            ou1 = ps_big.tile([128, 512], F32, tag="big", name=f"ou1_{i}")
            ou_t[i] = (ou0, ou1)
        ou0, ou1 = ou_t[i]
        ats = ats_t.pop(t)
        nc.tensor.matmul(ou0, ats, wosb_sb[:, h, 0:512],
                         start=(h == 0), stop=(h == 7))
        nc.tensor.matmul(ou1, ats, wosb_sb[:, h, 512:1024],
                         start=(h == 0), stop=(h == 7))

    def e_ob(i):
        # first half on DVE (emitted at slot start so the ou bank frees fast,
        # unblocking tile i+1's first out-proj matmul), second half on ACT
        ou0, ou1 = ou_t.pop(i)
        ob = ob_p.tile([128, D], F32, tag="ob", name=f"ob{i}")
        nc.vector.tensor_copy(ob[:, 0:512], ou0)
        nc.sync.dma_start(out_d[128 * i:128 * (i + 1), 0:512], ob[:, 0:512])
        nc.scalar.copy(ob[:, 512:1024], ou1)
        nc.sync.dma_start(out_d[128 * i:128 * (i + 1), 512:1024], ob[:, 512:1024])

    # ---- emission: projections with the phase-2 prologue overlapped into the
    # tail of phase 1 (softmax chain of tiles 0-1 runs while head 7 projects) ----
    _vhalf0_chunkmajor()
    _qk(0, fill=QK0_FILL)
    _vhalf(1)
    _qk(1)
    _qk(2)
    _qk(3)
    _qk(4)
    _qk(5)
    _qk(6)
    e_lg(0)
    e_exp(0)
    e_recip(0)
    e_mul(0)
    e_lg(1)
    _qk(7)
    e_exp(1)
    e_recip(1)
    e_mul(1)
    e_lg(2)
    e_exp(2)
    e_recip(2)
    e_mul(2)
    e_tr(0)
    e_sct(0)
    e_tr(1)
    e_at(0)
    e_sct(1)
    e_ats(0)
    e_lg(3)
    e_exp(3)
    e_recip(3)
    e_mul(3)
    e_tr(2)
    e_at(1)
    e_sct(2)
    e_ats(1)
    PRE_CHAIN, PRE_TR, PRE_AT = 4, 3, 2

    # pipeline, slot u: PE [tr(u-2), at(u-3), op(u-4), lg(u+2)],
    # ACT [exp(u), ob], DVE [sct(u-2), ats(u-3), recip(u)], Pool [mul(u)].
    # Chain lg(t)->exp(t)->recip(t)->mul(t) finishes mid slot t+1; tr(t) runs
    # slot t+2, so PE never waits on the softmax chain in steady state.
    for u in range(NT + 7):
        if PRE_TR <= u - 3 < NT:
            e_tr(u - 3)
        if PRE_AT <= u - 4 < NT:
            e_at(u - 4)
        if 0 <= u - 5 < NT:
            e_op(u - 5)
        if PRE_CHAIN <= u + 2 < NT:
            e_lg(u + 2)
        if u >= 13 and (u - 13) % 8 == 0 and (u - 13) // 8 < NQT:
            e_ob((u - 13) // 8)
        if PRE_CHAIN <= u < NT:
            e_exp(u)
        if PRE_TR <= u - 3 < NT:
            e_sct(u - 3)
        if PRE_AT <= u - 4 < NT:
            e_ats(u - 4)
        if PRE_CHAIN <= u < NT:
            e_recip(u)
            e_mul(u)


def _host_prep(x, Wq, bq, Wk, bk, Wv, bv, Wo, bo, dilations):
    import ml_dtypes
    f = np.float32
    bf = ml_dtypes.bfloat16
    x = np.asarray(x, f)
    x_pad = np.zeros((B, S + 2 * HALO, D), f)
    x_pad[:, HALO:HALO + S] = x

    Wo_s = np.asarray(Wo, f) * np.float32(E) ** f(-0.5)
    wvrb = np.ascontiguousarray(
        np.asarray(Wv, f).transpose(1, 0, 2).reshape(D, H * E)).astype(bf)
    wosb = np.ascontiguousarray(Wo_s).astype(bf)
    bqt = np.ascontiguousarray(np.asarray(bq, f).T)      # [E, H]

    # host-folded constant: sum_h (bv_h/sqrt(E)) @ Wo_h + bo  (sum of scores = 1)
    hostc = np.einsum('he,hed->d', np.asarray(bv, f) * np.float32(E) ** f(-0.5),
                      np.asarray(Wo, f)) + np.asarray(bo, f)

    dil = np.asarray(dilations).astype(np.int64)
    masks = np.full((H, 128, 256), MASKVAL, f)
    s_i = np.arange(128)[:, None]
    t_i = np.arange(256)[None, :]
    for h in range(H):
        d = int(dil[h])
        off = (d * (KW - 1)) // 2
        delta = t_i - s_i - HALO + off
        win = (delta >= 0) & (delta <= (KW - 1) * d) & (delta % d == 0)
        masks[h][win] = SHIFT
    mkb = masks.astype(bf)

    # q/k path in fp16, weights pre-rearranged to [H, 128, DC*E] (contiguous
    # >=512B DMA rows: [p, c, e] layout per head)
    wq16 = np.ascontiguousarray(
        np.asarray(Wq, f).reshape(H, DC, 128, E).transpose(0, 2, 1, 3)
        .reshape(H, 128, DC * E)).astype(np.float16)
    wk16 = np.ascontiguousarray(
        np.asarray(Wk, f).reshape(H, DC, 128, E).transpose(0, 2, 1, 3)
        .reshape(H, 128, DC * E)).astype(np.float16)

    shared = {
        "wq": wq16, "wk": wk16,
        "wvrb": wvrb, "wosb": wosb, "mkb": mkb, "bqt": bqt,
    }
    in_maps = []
    for c in range(NC_):
        b, idx = divmod(c, 4)
        xt = np.ascontiguousarray(
            x_pad[b, idx * CHUNK: idx * CHUNK + SPAN].T).astype(np.float16)
        in_maps.append({"xt": xt, **shared})
    return in_maps, hostc


def kernel(x, Wq, bq, Wk, bk, Wv, bv, Wo, bo, dilations):
    from concourse.bass_utils import run_bass_kernel_spmd

    if "nc" not in _CACHE:
        _CACHE["nc"] = _build_nc()
    nc = _CACHE["nc"]

    in_maps, hostc = _host_prep(x, Wq, bq, Wk, bk, Wv, bv, Wo, bo, dilations)
    res = run_bass_kernel_spmd(nc, in_maps, core_ids=list(range(NC_)))

    out = np.empty((B, S, D), np.float32)
    for c in range(NC_):
        b, idx = divmod(c, 4)
        out[b, idx * CHUNK:(idx + 1) * CHUNK] = res.results[c]["out"]
    out += hostc[None, None, :]
    return out
